# revision 5
# baseline (speedup 1.0000x reference)
"""AtomicNumberPooling Trainium2 kernel (v9 — mixed-path DMA'd one-hot).

Math (from the reference):
    keys   = batch * 100 + (z - 1)                    # per-node (graph, bin) id
    sums   = segment_sum(out, keys, G * 100)          # [G*100, D]
    counts = nodes per graph                          # [G]
    pooled = sums.reshape(G, 100 * D) / max(counts, 1)

Strategy: data-parallel over graphs — 64 graphs per NeuronCore, one
128-row chunk per graph (x pre-scaled by 1/count on host; >128-node
graphs get overflow chunks merged back on the host).  Per chunk:
onehot[128,100].T @ x[128,64] -> the graph's [100,64] block in PSUM.

The one-hot is precomputed on the host as uint8 (removing the ~7us of
DVE is_equal work that bottlenecked v6) and reaches SBUF two ways:
 - bulk middle pieces: SWDGE cast-DMA (uint8->bf16 in the DMA engines,
   zero compute-engine cost, issued from the otherwise idle GpSimd);
 - head and tail pieces: HWDGE raw uint8 + a DVE expand copy
   (u8->bf16 tensor_copy, ~62 ns/chunk HW-measured).  HWDGE semaphores
   fire ~1.0us after the data vs ~2.3us for SWDGE, so the stream starts
   earlier and the last matmul isn't held hostage by SWDGE latency.

The stream is input-DMA-wall bound (~1.9MB reads), so matmul pitch
stays the 87ns unpadded-weight rate (128-padding for FWL would add
bytes to the binding wall).  Other v9 scheduling choices:
 - big stores ride SWDGE from GpSimd: HWDGE dma_start instructions cost
   ~0.7us of Sync/Scalar queue time each, and those queues are the
   scarce resource;
 - big-group PSUM drains split at the bank boundary (512 f32 cols)
   between ScalarE and DVE, overlapping the stream;
 - PSUM groups in natural order with the tiny overflow group last: the
   post-stream tail is a ~250ns drain + small store on the fast ring,
   then only the ~2.4us HBM store receipt remains.
"""

import bisect

import numpy as np

NUM_Z = 100
G = 512
P = 128
NCORES = 8
GL = G // NCORES  # graphs per core
PB = 16           # chunks per PSUM tile (2 banks)

# filled by kernel() for optional inspection by a test harness
LAST_RESULTS = None


def _pieces(C, pattern):
    """Clip a piece-size pattern to C chunks."""
    out = []
    c = 0
    for n in pattern:
        if c >= C:
            break
        n = min(n, C - c)
        out.append((c, n))
        c += n
    while c < C:  # pattern exhausted: extend with 16s
        n = min(16, C - c)
        out.append((c, n))
        c += n
    return out


def _x_plan(C):
    return _pieces(C, [1, 3, 12, 16, 16, 12, 4, 1])


def _oh_plan(C):
    """(c0, cn, via_hwdge) pieces.  Head and tail via HWDGE+DVE-expand
    (fast sems), middle bulk via SWDGE cast (no engine cost)."""
    sizes = _pieces(C, [4, 12, 16, 16, 12, 4, 1])
    n = len(sizes)
    out = []
    for i, (c0, cn) in enumerate(sizes):
        hw = i < 2 or i >= n - 2
        out.append((c0, cn, hw))
    return out


def _build_program(C, D):
    import concourse.bacc as bacc
    import concourse.mybir as mybir
    import concourse.tile as tile

    f32 = mybir.dt.float32
    bf16 = mybir.dt.bfloat16
    u8 = mybir.dt.uint8
    nc = bacc.Bacc("TRN2", debug=False, num_devices=NCORES)

    xps = _x_plan(C)
    ops = _oh_plan(C)
    x_d = nc.dram_tensor("x", [P, C * D], bf16, kind="ExternalInput")
    oh_d = nc.dram_tensor("oh", [P, C * NUM_Z], u8, kind="ExternalInput")
    y_d = nc.dram_tensor("y", [NUM_Z, C * D], bf16, kind="ExternalOutput")

    NGRP = (C + PB - 1) // PB
    last_cn = C - (NGRP - 1) * PB

    with tile.TileContext(nc) as tc:
        with (
            tc.tile_pool(name="xin", bufs=1) as xp,
            tc.tile_pool(name="oh", bufs=1) as ohp,
            tc.tile_pool(name="stage", bufs=NGRP) as stp,
            tc.tile_pool(name="psum", bufs=4, space="PSUM") as pp,
        ):
            # x pieces alternate the two HWDGE rings
            xts = []
            for i, (c0, cn) in enumerate(xps):
                xt = xp.tile([P, cn * D], bf16, name=f"x{i}")
                eng = nc.sync if i % 2 == 0 else nc.scalar
                eng.dma_start(xt[:], x_d[:, c0 * D : (c0 + cn) * D])
                xts.append(xt)

            # one-hot pieces
            ohts = []
            hw_i = 0
            for i, (c0, cn, hw) in enumerate(ops):
                oht = ohp.tile([P, cn * NUM_Z], bf16, name=f"oh{i}")
                if hw:
                    ut = ohp.tile([P, cn * NUM_Z], u8, name=f"u{i}")
                    eng = nc.scalar if hw_i % 2 == 0 else nc.sync
                    hw_i += 1
                    eng.dma_start(
                        ut[:], oh_d[:, c0 * NUM_Z : (c0 + cn) * NUM_Z]
                    )
                    # expand in <=4-chunk slices so the first matmuls of
                    # the piece aren't gated on the whole piece's expand
                    for s0 in range(0, cn, 4):
                        sn = min(4, cn - s0)
                        nc.vector.tensor_copy(
                            oht[:, s0 * NUM_Z : (s0 + sn) * NUM_Z],
                            ut[:, s0 * NUM_Z : (s0 + sn) * NUM_Z],
                        )
                else:
                    nc.gpsimd.dma_start(
                        oht[:], oh_d[:, c0 * NUM_Z : (c0 + cn) * NUM_Z]
                    )
                ohts.append(oht)

            x_starts = [s for s, _ in xps]
            oh_starts = [s for s, _, _ in ops]

            # matmul stream + per-psum-group drain/store, natural order
            for g in range(NGRP):
                c0 = g * PB
                cn = min(PB, C - c0)
                ps = pp.tile([P, cn * D], f32)
                for jj in range(cn):
                    j = c0 + jj
                    xi = bisect.bisect_right(x_starts, j) - 1
                    oi = bisect.bisect_right(oh_starts, j) - 1
                    xs0 = xps[xi][0]
                    os0 = ops[oi][0]
                    nc.tensor.matmul(
                        out=ps[:NUM_Z, jj * D : (jj + 1) * D],
                        lhsT=ohts[oi][
                            :, (j - os0) * NUM_Z : (j - os0 + 1) * NUM_Z
                        ],
                        rhs=xts[xi][:, (j - xs0) * D : (j - xs0 + 1) * D],
                        start=True,
                        stop=True,
                    )
                stage = stp.tile([P, cn * D], bf16, name=f"st{g}")
                cols = cn * D
                if cols > 512:
                    # split at the PSUM bank boundary: ScalarE drains
                    # bank 0 while bank 1 is still accumulating
                    h = 512
                    nc.scalar.copy(stage[:NUM_Z, :h], ps[:NUM_Z, :h])
                    nc.vector.tensor_copy(
                        stage[:NUM_Z, h:cols], ps[:NUM_Z, h:cols]
                    )
                else:
                    nc.scalar.copy(stage[:NUM_Z, :cols], ps[:NUM_Z, :cols])
                if g == NGRP - 1 or (g == NGRP - 2 and last_cn <= 4):
                    # receipt-critical late stores go on the fast HWDGE
                    # rings, split so neither ring serializes them
                    if cols > 512:
                        h2 = cols // 2
                        nc.sync.dma_start(
                            y_d[:, c0 * D : c0 * D + h2],
                            stage[:NUM_Z, :h2],
                        )
                        nc.scalar.dma_start(
                            y_d[:, c0 * D + h2 : c0 * D + cols],
                            stage[:NUM_Z, h2:cols],
                        )
                    else:
                        nc.sync.dma_start(
                            y_d[:, c0 * D : c0 * D + cols],
                            stage[:NUM_Z, :cols],
                        )
                else:
                    # mid-stream stores ride SWDGE from the idle GpSimd,
                    # sparing the scarce Sync/Scalar queue time
                    nc.gpsimd.dma_start(
                        y_d[:, c0 * D : c0 * D + cols], stage[:NUM_Z, :cols]
                    )
    nc.compile()
    return nc


def _prep(x, z, b, D):
    """Build per-core padded inputs.  Returns (in_maps, over_maps, C)."""
    import ml_dtypes

    counts = np.bincount(b, minlength=G).astype(np.int64)
    starts = np.zeros(G + 1, np.int64)
    np.cumsum(counts, out=starts[1:])
    inv = 1.0 / np.maximum(counts, 1).astype(np.float32)
    xs = (x * inv[b][:, None]).astype(ml_dtypes.bfloat16)

    per_core = []
    for k in range(NCORES):
        main = []  # (node_start, length, graph) — one per graph, in order
        over = []  # extra pieces for graphs with >P nodes
        for gl in range(GL):
            g = k * GL + gl
            s, n = int(starts[g]), int(counts[g])
            main.append((s, min(n, P), g))
            off = P
            while off < n:
                over.append((s + off, min(n - off, P), g))
                off += P
        per_core.append((main, over))

    B = max(len(o) for _, o in per_core)
    C = GL + B

    in_maps, over_maps = [], []
    for k in range(NCORES):
        main, over = per_core[k]
        chunks = main + over
        xT = np.zeros((P, C, D), ml_dtypes.bfloat16)
        zb = np.full((P, C), -1, np.int64)
        for j, (s, ln, g) in enumerate(chunks):
            xT[:ln, j, :] = xs[s : s + ln]
            zb[:ln, j] = z[s : s + ln]
        oh = np.zeros((P, C, NUM_Z), np.uint8)
        rr, jj = np.nonzero(zb >= 0)
        oh[rr, jj, zb[rr, jj]] = 1
        in_maps.append(
            {
                "x": np.ascontiguousarray(xT.reshape(P, C * D)),
                "oh": np.ascontiguousarray(oh.reshape(P, C * NUM_Z)),
            }
        )
        over_maps.append([(GL + j, g) for j, (s, ln, g) in enumerate(over)])
    return in_maps, over_maps, C


def _ensure_ntff_hook():
    """run_bass_kernel_spmd(trace=True) under axon imports antenv.axon_hooks,
    which this agent image lacks — recreate it (with the ctypes NTFF hook if
    available) so a BASS_TRACE=1 environment doesn't crash kernel()."""
    import sys
    import types

    try:
        import antenv.axon_hooks  # noqa: F401

        return
    except ImportError:
        pass
    try:
        import antenv
    except ImportError:
        return
    hook = None
    try:
        from trn_agent_boot.trn_boot import _ntff_profile_via_ctypes

        hook = _ntff_profile_via_ctypes("/opt/axon/libaxon_pjrt.so")
    except Exception:
        pass
    mod = types.ModuleType("antenv.axon_hooks")
    mod._hook = hook
    mod.get_axon_ntff_profile_hook = lambda: mod._hook
    mod.set_axon_ntff_profile_hook = lambda h: setattr(mod, "_hook", h)
    sys.modules["antenv.axon_hooks"] = mod
    antenv.axon_hooks = mod


def kernel(out, z_rv, x_rv_batch):
    global LAST_RESULTS
    from concourse.bass_utils import run_bass_kernel_spmd

    _ensure_ntff_hook()

    x = np.ascontiguousarray(np.asarray(out), dtype=np.float32)
    z = np.asarray(z_rv).astype(np.int64) - 1  # 0..99
    b = np.asarray(x_rv_batch).astype(np.int64)
    D = x.shape[1]

    in_maps, over_maps, C = _prep(x, z, b, D)
    nc = _build_program(C, D)
    res = run_bass_kernel_spmd(nc, in_maps, core_ids=list(range(NCORES)))
    LAST_RESULTS = res

    full = np.empty((G, NUM_Z * D), np.float32)
    for k in range(NCORES):
        yk = np.asarray(res.results[k]["y"]).astype(np.float32)
        blocks = (
            yk.reshape(NUM_Z, C, D).transpose(1, 0, 2).reshape(C, NUM_Z * D)
        )
        full[k * GL : (k + 1) * GL] = blocks[:GL]
        for j, g in over_maps[k]:
            full[g] += blocks[j]
    return full


# revision 6
# speedup vs baseline: 1.2000x; 1.2000x over previous
"""AtomicNumberPooling Trainium2 kernel (v10 — HWDGE uint8 one-hot, DVE expand).

Math (from the reference):
    keys   = batch * 100 + (z - 1)                    # per-node (graph, bin) id
    sums   = segment_sum(out, keys, G * 100)          # [G*100, D]
    counts = nodes per graph                          # [G]
    pooled = sums.reshape(G, 100 * D) / max(counts, 1)

Strategy: data-parallel over graphs — 64 graphs per NeuronCore, one
128-row chunk per graph (x pre-scaled by 1/count on host; >128-node
graphs get overflow chunks merged back on the host).  Per chunk:
onehot[128,100].T @ x[128,64] -> the graph's [100,64] block in PSUM.

The one-hot is precomputed on the host as uint8 (kills the ~7us of DVE
is_equal work that bottlenecked v6 at an 8.5us stream) and loaded raw
over the two HWDGE rings, then expanded u8->bf16 by DVE tensor_copy
(~70 ns/chunk HW-measured, 2x perf mode).  HWDGE everywhere because
SWDGE completion semaphores fire ~2.3us after the data lands (HWDGE
~1.0us) — measured on v8/v9 — and any mid-stream gate that late stalls
the whole 87 ns/chunk matmul pipeline.

Scheduling (from v8/v9 traces):
 - loads are EMITTED in consumption order, interleaved x/oh on both
   rings: HWDGE rings are FIFO, so emission order = data-arrival order
   (v9 emitted oh after all x and the first matmul slipped 8us);
 - piece-size tails shrink to 1 chunk so the last matmul trails the
   ~1.9MB input DMA wall by only the ~1.0us HWDGE sem latency;
 - mid-stream stores ride SWDGE from the idle GpSimd (saves ~0.7us of
   Sync/Scalar queue time each; their receipts are not on the critical
   path), the receipt-critical last stores go HWDGE, split across
   rings;
 - PSUM drains: early groups coarse on ScalarE, late groups split at
   the bank boundary between ScalarE and DVE so the tail is short;
 - PSUM groups in natural order: the tiny overflow group is last, so
   after the final matmul only ~0.5us of drain+store issue remains
   before the (fixed) ~2.4us HBM store receipt.
"""

import bisect

import numpy as np

NUM_Z = 100
G = 512
P = 128
NCORES = 8
GL = G // NCORES  # graphs per core
PB = 16           # chunks per PSUM tile (2 banks)

# filled by kernel() for optional inspection by a test harness
LAST_RESULTS = None


def _pieces(C, pattern):
    out = []
    c = 0
    for n in pattern:
        if c >= C:
            break
        n = min(n, C - c)
        out.append((c, n))
        c += n
    while c < C:
        n = min(16, C - c)
        out.append((c, n))
        c += n
    return out


def _x_plan(C):
    return _pieces(C, [1, 3, 12, 16, 16, 16, 1])


def _oh_plan(C):
    return _pieces(C, [4, 12, 16, 16, 16, 1])


def _build_program(C, D):
    import concourse.bacc as bacc
    import concourse.mybir as mybir
    import concourse.tile as tile

    f32 = mybir.dt.float32
    bf16 = mybir.dt.bfloat16
    u8 = mybir.dt.uint8
    nc = bacc.Bacc("TRN2", debug=False, num_devices=NCORES)

    xps = _x_plan(C)
    ops = _oh_plan(C)
    x_d = nc.dram_tensor("x", [P, C * D], bf16, kind="ExternalInput")
    oh_d = nc.dram_tensor("oh", [P, C * NUM_Z], u8, kind="ExternalInput")
    y_d = nc.dram_tensor("y", [NUM_Z, C * D], bf16, kind="ExternalOutput")

    NGRP = (C + PB - 1) // PB
    last_cn = C - (NGRP - 1) * PB

    with tile.TileContext(nc) as tc:
        with (
            tc.tile_pool(name="xin", bufs=1) as xp,
            tc.tile_pool(name="oh", bufs=1) as ohp,
            tc.tile_pool(name="stage", bufs=NGRP) as stp,
            tc.tile_pool(name="psum", bufs=4, space="PSUM") as pp,
        ):
            # merge x and oh pieces into one consumption-ordered load
            # list; alternate rings so each ring's FIFO receives its
            # pieces in the order the matmul stream needs them.
            loads = []  # (start_chunk, kind, piece_idx)
            for i, (c0, cn) in enumerate(xps):
                loads.append((c0, 0, i))
            for i, (c0, cn) in enumerate(ops):
                loads.append((c0, 1, i))
            loads.sort(key=lambda t: (t[0], t[1]))

            xts = [None] * len(xps)
            uts = [None] * len(ops)
            ohts = [None] * len(ops)
            for ring, (c0, kind, i) in enumerate(loads):
                eng = nc.sync if ring % 2 == 0 else nc.scalar
                if kind == 0:
                    cn = xps[i][1]
                    xt = xp.tile([P, cn * D], bf16, name=f"x{i}")
                    eng.dma_start(xt[:], x_d[:, c0 * D : (c0 + cn) * D])
                    xts[i] = xt
                else:
                    cn = ops[i][1]
                    ut = ohp.tile([P, cn * NUM_Z], u8, name=f"u{i}")
                    eng.dma_start(
                        ut[:], oh_d[:, c0 * NUM_Z : (c0 + cn) * NUM_Z]
                    )
                    uts[i] = ut

            # DVE expands u8 -> bf16 in <=4-chunk slices, in consumption
            # order, so the stream is never gated on a whole piece
            for i, (c0, cn) in enumerate(ops):
                oht = ohp.tile([P, cn * NUM_Z], bf16, name=f"oh{i}")
                for s0 in range(0, cn, 4):
                    sn = min(4, cn - s0)
                    nc.vector.tensor_copy(
                        oht[:, s0 * NUM_Z : (s0 + sn) * NUM_Z],
                        uts[i][:, s0 * NUM_Z : (s0 + sn) * NUM_Z],
                    )
                ohts[i] = oht

            x_starts = [s for s, _ in xps]
            oh_starts = [s for s, _ in ops]

            # matmul stream + per-psum-group drain/store, natural order
            for g in range(NGRP):
                c0 = g * PB
                cn = min(PB, C - c0)
                ps = pp.tile([P, cn * D], f32)
                for jj in range(cn):
                    j = c0 + jj
                    xi = bisect.bisect_right(x_starts, j) - 1
                    oi = bisect.bisect_right(oh_starts, j) - 1
                    xs0 = xps[xi][0]
                    os0 = ops[oi][0]
                    nc.tensor.matmul(
                        out=ps[:NUM_Z, jj * D : (jj + 1) * D],
                        lhsT=ohts[oi][
                            :, (j - os0) * NUM_Z : (j - os0 + 1) * NUM_Z
                        ],
                        rhs=xts[xi][:, (j - xs0) * D : (j - xs0 + 1) * D],
                        start=True,
                        stop=True,
                    )
                stage = stp.tile([P, cn * D], bf16, name=f"st{g}")
                cols = cn * D
                late = g >= NGRP - 2
                if cols > 512 and late:
                    # tail groups: split at the PSUM bank boundary between
                    # ScalarE and DVE so the post-stream drain is short
                    h = 512
                    nc.scalar.copy(stage[:NUM_Z, :h], ps[:NUM_Z, :h])
                    nc.vector.tensor_copy(
                        stage[:NUM_Z, h:cols], ps[:NUM_Z, h:cols]
                    )
                else:
                    nc.scalar.copy(stage[:NUM_Z, :cols], ps[:NUM_Z, :cols])
                if late:
                    # receipt-critical stores go HWDGE, split across rings
                    if cols > 512:
                        h2 = cols // 2
                        nc.sync.dma_start(
                            y_d[:, c0 * D : c0 * D + h2],
                            stage[:NUM_Z, :h2],
                        )
                        nc.scalar.dma_start(
                            y_d[:, c0 * D + h2 : c0 * D + cols],
                            stage[:NUM_Z, h2:cols],
                        )
                    else:
                        nc.sync.dma_start(
                            y_d[:, c0 * D : c0 * D + cols],
                            stage[:NUM_Z, :cols],
                        )
                else:
                    # mid-stream stores ride SWDGE from the idle GpSimd
                    nc.gpsimd.dma_start(
                        y_d[:, c0 * D : c0 * D + cols], stage[:NUM_Z, :cols]
                    )
    nc.compile()
    return nc


def _prep(x, z, b, D):
    """Build per-core padded inputs.  Returns (in_maps, over_maps, C)."""
    import ml_dtypes

    counts = np.bincount(b, minlength=G).astype(np.int64)
    starts = np.zeros(G + 1, np.int64)
    np.cumsum(counts, out=starts[1:])
    inv = 1.0 / np.maximum(counts, 1).astype(np.float32)
    xs = (x * inv[b][:, None]).astype(ml_dtypes.bfloat16)

    per_core = []
    for k in range(NCORES):
        main = []  # (node_start, length, graph) — one per graph, in order
        over = []  # extra pieces for graphs with >P nodes
        for gl in range(GL):
            g = k * GL + gl
            s, n = int(starts[g]), int(counts[g])
            main.append((s, min(n, P), g))
            off = P
            while off < n:
                over.append((s + off, min(n - off, P), g))
                off += P
        per_core.append((main, over))

    B = max(len(o) for _, o in per_core)
    C = GL + B

    in_maps, over_maps = [], []
    for k in range(NCORES):
        main, over = per_core[k]
        chunks = main + over
        xT = np.zeros((P, C, D), ml_dtypes.bfloat16)
        zb = np.full((P, C), -1, np.int64)
        for j, (s, ln, g) in enumerate(chunks):
            xT[:ln, j, :] = xs[s : s + ln]
            zb[:ln, j] = z[s : s + ln]
        oh = np.zeros((P, C, NUM_Z), np.uint8)
        rr, jj = np.nonzero(zb >= 0)
        oh[rr, jj, zb[rr, jj]] = 1
        in_maps.append(
            {
                "x": np.ascontiguousarray(xT.reshape(P, C * D)),
                "oh": np.ascontiguousarray(oh.reshape(P, C * NUM_Z)),
            }
        )
        over_maps.append([(GL + j, g) for j, (s, ln, g) in enumerate(over)])
    return in_maps, over_maps, C


def _ensure_ntff_hook():
    """run_bass_kernel_spmd(trace=True) under axon imports antenv.axon_hooks,
    which this agent image lacks — recreate it (with the ctypes NTFF hook if
    available) so a BASS_TRACE=1 environment doesn't crash kernel()."""
    import sys
    import types

    try:
        import antenv.axon_hooks  # noqa: F401

        return
    except ImportError:
        pass
    try:
        import antenv
    except ImportError:
        return
    hook = None
    try:
        from trn_agent_boot.trn_boot import _ntff_profile_via_ctypes

        hook = _ntff_profile_via_ctypes("/opt/axon/libaxon_pjrt.so")
    except Exception:
        pass
    mod = types.ModuleType("antenv.axon_hooks")
    mod._hook = hook
    mod.get_axon_ntff_profile_hook = lambda: mod._hook
    mod.set_axon_ntff_profile_hook = lambda h: setattr(mod, "_hook", h)
    sys.modules["antenv.axon_hooks"] = mod
    antenv.axon_hooks = mod


def kernel(out, z_rv, x_rv_batch):
    global LAST_RESULTS
    from concourse.bass_utils import run_bass_kernel_spmd

    _ensure_ntff_hook()

    x = np.ascontiguousarray(np.asarray(out), dtype=np.float32)
    z = np.asarray(z_rv).astype(np.int64) - 1  # 0..99
    b = np.asarray(x_rv_batch).astype(np.int64)
    D = x.shape[1]

    in_maps, over_maps, C = _prep(x, z, b, D)
    nc = _build_program(C, D)
    res = run_bass_kernel_spmd(nc, in_maps, core_ids=list(range(NCORES)))
    LAST_RESULTS = res

    full = np.empty((G, NUM_Z * D), np.float32)
    for k in range(NCORES):
        yk = np.asarray(res.results[k]["y"]).astype(np.float32)
        blocks = (
            yk.reshape(NUM_Z, C, D).transpose(1, 0, 2).reshape(C, NUM_Z * D)
        )
        full[k * GL : (k + 1) * GL] = blocks[:GL]
        for j, g in over_maps[k]:
            full[g] += blocks[j]
    return full
